# revision 4
# baseline (speedup 1.0000x reference)
"""Trainium2 Bass kernel for a two-branch sparse message-passing layer (CAN layer).

Computes, given COO edge lists and dense 128x128 weights:
  y0 = relu(spmm(A00, relu(x_0) @ W0))      A00: [100000,100000], 1.6M nnz
  y1 = relu(x_1)                            passthrough [150000,128]
  y2 = relu(spmm(B12, relu(x_1) @ W12))     B12: [50000,150000], 200K nnz

Strategy: shard destination rows across 8 NeuronCores. Each core:
  - gathers source rows for its edges via dma_gather (int16 windowed indices),
  - applies relu on the gathered tiles,
  - aggregates per 128-destination block with one-hot scatter matmuls
    (S[e, d] = (iota_d == dest_local_e) * val_e built on DVE, accumulated
    in fp32 PSUM by the TensorEngine; uses A@(xW) = (A@x)@W associativity so
    the dense weight is applied once per aggregated output block in fp32),
  - applies the dense weight + relu and writes its output shard.

The gather tables and the scatter matmuls run in bf16 (fp32 accumulation);
the x_1 relu passthrough and the dense-weight stage stay fp32.

Host side only shards/organizes the COO lists and concatenates shards.
"""
import math
import numpy as np
import ml_dtypes
from contextlib import ExitStack

import concourse.bass as bass
import concourse.tile as tile
from concourse import bacc, mybir
from concourse.bass_utils import run_bass_kernel_spmd

P = 128
C = 8                       # cores
N0, N1, N2 = 100000, 150000, 50000
R0, R1, R2 = N0 // C, N1 // C, N2 // C      # 12500, 18750, 6250
NB0 = math.ceil(R0 / P)     # 98 dest blocks/core, last has 84 rows
NB2 = math.ceil(R2 / P)     # 49 dest blocks/core, last has 106 rows
SB0 = 4                     # blocks per superblock (gather granularity)
SB2 = 7
W0SZ, NW0 = 25000, 4        # source windows (int16 index limit)
W2SZ, NW2 = 30000, 5
X1CH = 10                   # x1 relu chunks per core
X1W = R1 // X1CH            # 1875 free-dim elements per chunk

USE_BF16 = True             # bf16 gathers + scatter matmuls (fp32 PSUM/dense)

F32 = mybir.dt.float32
BF16 = mybir.dt.bfloat16
I16 = mybir.dt.int16
DT_G = BF16 if USE_BF16 else F32
NP_G = ml_dtypes.bfloat16 if USE_BF16 else np.float32


class _Branch:
    """Host-side slot layout for one SpMM branch (shared across cores)."""

    def __init__(self, rows, cols, vals, R, nb, sbsz, wsz, nw):
        nsb = math.ceil(nb / sbsz)
        core = rows // R
        local = rows - core * R
        b = local // P
        dl = (local % P).astype(np.float32)
        q = cols // wsz
        qcol = (cols % wsz).astype(np.int16)
        sb = b // sbsz
        b_in = b % sbsz

        ncell = nsb * nw * sbsz
        cell = (sb * nw + q) * sbsz + b_in
        key = core * ncell + cell
        cnt = np.bincount(key, minlength=C * ncell).reshape(C, nsb, nw, sbsz)
        g = np.ceil(cnt.max(axis=0) / P).astype(np.int64)   # [nsb, nw, sbsz]

        # group-id offsets within each sb (q-major, then block)
        gflat = g.reshape(nsb, nw * sbsz)
        ggoff = np.zeros((nsb, nw * sbsz), np.int64)
        ggoff[:, 1:] = np.cumsum(gflat, axis=1)[:, :-1]
        ggoff = ggoff.reshape(nsb, nw, sbsz)
        G_sb = gflat.sum(axis=1)                            # groups per sb
        gq = g.sum(axis=2)                                  # [nsb, nw]
        q_goff = ggoff[:, :, 0]                             # first group of (sb,q)

        # flat offsets into the metadata tensors
        ioff = np.zeros((nsb, nw), np.int64)                # int16 elements
        np.cumsum((P * 8 * gq).ravel()[:-1], out=ioff.ravel()[1:])
        TI = int(P * 8 * gq.sum())
        vdoff = np.zeros(nsb, np.int64)
        np.cumsum((P * 2 * G_sb)[:-1], out=vdoff[1:])
        TV = int(P * 2 * G_sb.sum())

        # per-edge slot assignment
        order = np.lexsort((b_in, q, sb, core))
        ks = key[order]
        starts = np.r_[0, np.flatnonzero(np.diff(ks)) + 1]
        seg_len = np.diff(np.r_[starts, len(ks)])
        rank = np.arange(len(ks)) - np.repeat(starts, seg_len)

        core_s, sb_s, q_s, bin_s = core[order], sb[order], q[order], b_in[order]
        p_s = rank % P
        gg_s = ggoff[sb_s, q_s, bin_s] + rank // P
        j = (gg_s - q_goff[sb_s, q_s]) * P + p_s
        gq_s = gq[sb_s, q_s]

        idx_all = np.zeros((C, max(TI, 1)), np.int16)
        vd_all = np.zeros((C, max(TV, 1)), np.float32)
        base_i = ioff[sb_s, q_s]
        colw = j // 16
        r16 = j % 16
        qcol_s = qcol[order]
        for rr in range(8):
            pos = base_i + (16 * rr + r16) * (8 * gq_s) + colw
            idx_all[core_s, pos] = qcol_s
        G_s = G_sb[sb_s]
        posv = vdoff[sb_s] + p_s * (2 * G_s) + gg_s
        vd_all[core_s, posv] = vals[order]
        vd_all[core_s, posv + G_s] = dl[order]

        self.nb, self.sbsz, self.nsb, self.nw, self.wsz = nb, sbsz, nsb, nw, wsz
        self.R = R
        self.g, self.ggoff, self.G_sb, self.gq = g, ggoff, G_sb, gq
        self.ioff, self.vdoff, self.TI, self.TV = ioff, vdoff, TI, TV
        self.idx_all, self.vd_all = idx_all, vd_all

    def sb_blocks(self, s):
        return range(s * self.sbsz, min((s + 1) * self.sbsz, self.nb))


def _emit_branch_sb(nc, ctx, pools, br, s, src_h, idx_h, vd_h, w_t, iota_t, y_h):
    """Emit one superblock of a SpMM branch."""
    xg_pool, meta_pool, s_pool, t_pool, out_pool, ps1, ps2 = pools
    G_sb = int(br.G_sb[s])
    if G_sb == 0:
        return
    vd_t = meta_pool.tile([P, 2 * G_sb], F32, tag=f"vd{br.wsz}")
    nc.sync.dma_start(vd_t[:], bass.AP(vd_h, int(br.vdoff[s]), [[2 * G_sb, P], [1, 2 * G_sb]]))

    xg = xg_pool.tile([P, G_sb * P], DT_G, tag=f"xg{br.wsz}")
    for q in range(br.nw):
        gq = int(br.gq[s, q])
        if gq == 0:
            continue
        nidx = P * gq
        idx_t = meta_pool.tile([P, 8 * gq], I16, tag=f"ix{br.wsz}")
        nc.sync.dma_start(idx_t[:], bass.AP(idx_h, int(br.ioff[s, q]), [[8 * gq, P], [1, 8 * gq]]))
        off = int(br.ggoff[s, q, 0])
        seg = xg[:, off * P:(off + gq) * P]
        nc.gpsimd.dma_gather(
            out_ap=seg.rearrange("p (g e) -> p g e", e=P),
            in_ap=src_h.ap()[q * br.wsz:(q + 1) * br.wsz, :],
            idxs_ap=idx_t[:],
            num_idxs=nidx,
            num_idxs_reg=nidx,
            elem_size=P,
            single_packet=(nidx <= 1024),
        )
        # relu the freshly gathered segment in place
        nc.scalar.activation(seg, seg, mybir.ActivationFunctionType.Relu)

    for b in br.sb_blocks(s):
        b_in = b - s * br.sbsz
        ggs = []
        for q in range(br.nw):
            o = int(br.ggoff[s, q, b_in])
            ggs.extend(range(o, o + int(br.g[s, q, b_in])))
        if not ggs:
            continue
        psum1 = ps1.tile([P, P], F32, space="PSUM", tag="ps1")
        for i, gg in enumerate(ggs):
            s_t = s_pool.tile([P, P], DT_G, tag="s")
            nc.vector.tensor_scalar(
                out=s_t[:], in0=iota_t[:],
                scalar1=vd_t[:, G_sb + gg:G_sb + gg + 1],
                scalar2=vd_t[:, gg:gg + 1],
                op0=mybir.AluOpType.is_equal, op1=mybir.AluOpType.mult,
            )
            nc.tensor.matmul(
                psum1[:], lhsT=xg[:, gg * P:(gg + 1) * P], rhs=s_t[:],
                start=(i == 0), stop=(i == len(ggs) - 1),
            )
        t1 = t_pool.tile([P, P], F32, tag="t1")
        nc.scalar.activation(t1[:], psum1[:], mybir.ActivationFunctionType.Copy)
        psum2 = ps2.tile([P, P], F32, space="PSUM", tag="ps2")
        nc.tensor.matmul(psum2[:], lhsT=t1[:], rhs=w_t[:], start=True, stop=True)
        yt = out_pool.tile([P, P], F32, tag="yt")
        nc.scalar.activation(yt[:], psum2[:], mybir.ActivationFunctionType.Relu)
        r0 = b * P
        nrow = min(br.R, r0 + P) - r0
        nc.sync.dma_start(y_h.ap()[r0:r0 + nrow, :], yt[:nrow, :])


def _build_program(br0, br2):
    nc = bacc.Bacc("TRN2", target_bir_lowering=False, debug=False, num_devices=C)
    x0_h = nc.dram_tensor("x0", [N0, P], DT_G, kind="ExternalInput")
    x1f_h = nc.dram_tensor("x1f", [N1, P], DT_G, kind="ExternalInput")
    x1s_h = nc.dram_tensor("x1s", [R1, P], F32, kind="ExternalInput")
    w0_h = nc.dram_tensor("w0", [P, P], F32, kind="ExternalInput")
    w12_h = nc.dram_tensor("w12", [P, P], F32, kind="ExternalInput")
    idx0_h = nc.dram_tensor("idx0", [max(br0.TI, 1)], I16, kind="ExternalInput")
    vd0_h = nc.dram_tensor("vd0", [max(br0.TV, 1)], F32, kind="ExternalInput")
    idx2_h = nc.dram_tensor("idx2", [max(br2.TI, 1)], I16, kind="ExternalInput")
    vd2_h = nc.dram_tensor("vd2", [max(br2.TV, 1)], F32, kind="ExternalInput")
    tok_h = nc.dram_tensor("tok", [1, P], F32, kind="ExternalInput")
    y0_h = nc.dram_tensor("y0", [R0, P], F32, kind="ExternalOutput")
    y1_h = nc.dram_tensor("y1", [R1, P], F32, kind="ExternalOutput")
    y2_h = nc.dram_tensor("y2", [R2, P], F32, kind="ExternalOutput")
    tko_h = nc.dram_tensor("tok_out", [1, P], F32, kind="ExternalOutput")

    with tile.TileContext(nc) as tc:
        with ExitStack() as ctx:
            const = ctx.enter_context(tc.tile_pool(name="const", bufs=1))
            xg_pool = ctx.enter_context(tc.tile_pool(name="xg", bufs=2))
            meta_pool = ctx.enter_context(tc.tile_pool(name="meta", bufs=3))
            s_pool = ctx.enter_context(tc.tile_pool(name="s", bufs=4))
            t_pool = ctx.enter_context(tc.tile_pool(name="t1", bufs=3))
            out_pool = ctx.enter_context(tc.tile_pool(name="out", bufs=3))
            x1_pool = ctx.enter_context(tc.tile_pool(name="x1", bufs=3))
            ps1 = ctx.enter_context(tc.tile_pool(name="ps1", bufs=4, space="PSUM"))
            ps2 = ctx.enter_context(tc.tile_pool(name="ps2", bufs=4, space="PSUM"))
            pools = (xg_pool, meta_pool, s_pool, t_pool, out_pool, ps1, ps2)

            # token roundtrip (used by the timing rig to chain executions)
            tok_t = const.tile([1, P], F32)
            nc.sync.dma_start(tok_t[:], tok_h.ap())
            nc.sync.dma_start(tko_h.ap(), tok_t[:])

            iota_t = const.tile([P, P], DT_G)
            nc.gpsimd.iota(iota_t[:], [[1, P]], channel_multiplier=0,
                           allow_small_or_imprecise_dtypes=True)
            w0_t = const.tile([P, P], F32)
            nc.sync.dma_start(w0_t[:], w0_h.ap())
            w12_t = const.tile([P, P], F32)
            nc.sync.dma_start(w12_t[:], w12_h.ap())

            # interleave work units for overlap: b0 superblocks, b2 superblocks,
            # x1 relu chunks
            units = [("b0", s) for s in range(br0.nsb)]
            step0 = max(1, br0.nsb // br2.nsb)
            for i in range(br2.nsb):
                units.insert(min(len(units), (i + 1) * (step0 + 1) - 1), ("b2", i))
            stepx = max(1, len(units) // X1CH)
            for i in range(X1CH):
                units.insert(min(len(units), (i + 1) * (stepx + 1) - 1), ("x1", i))

            for kind, s in units:
                if kind == "b0":
                    _emit_branch_sb(nc, ctx, pools, br0, s, x0_h, idx0_h, vd0_h,
                                    w0_t, iota_t, y0_h)
                elif kind == "b2":
                    _emit_branch_sb(nc, ctx, pools, br2, s, x1f_h, idx2_h, vd2_h,
                                    w12_t, iota_t, y2_h)
                else:
                    c = s
                    xt = x1_pool.tile([P, X1W], F32, tag="x1c")
                    src = bass.AP(x1s_h, c * X1W, [[R1, P], [1, X1W]])
                    dst = bass.AP(y1_h, c * X1W, [[R1, P], [1, X1W]])
                    nc.sync.dma_start(xt[:], src)
                    nc.scalar.activation(xt[:], xt[:], mybir.ActivationFunctionType.Relu)
                    nc.sync.dma_start(dst, xt[:])
    nc.compile()
    return nc


def _prepare(x_0, x_1, n00_rows, n00_cols, n00_vals, n12_rows, n12_cols, n12_vals,
             W0, W12):
    br0 = _Branch(np.asarray(n00_rows), np.asarray(n00_cols),
                  np.asarray(n00_vals, np.float32), R0, NB0, SB0, W0SZ, NW0)
    br2 = _Branch(np.asarray(n12_rows), np.asarray(n12_cols),
                  np.asarray(n12_vals, np.float32), R2, NB2, SB2, W2SZ, NW2)
    x_0g = np.ascontiguousarray(np.asarray(x_0).astype(NP_G))
    x_1 = np.ascontiguousarray(np.asarray(x_1, np.float32))
    x_1g = np.ascontiguousarray(x_1.astype(NP_G))
    tok = np.zeros((1, P), np.float32)
    in_maps = []
    for c in range(C):
        in_maps.append({
            "x0": x_0g,
            "x1f": x_1g,
            "x1s": np.ascontiguousarray(x_1[c * R1:(c + 1) * R1]),
            "w0": np.ascontiguousarray(np.asarray(W0, np.float32)),
            "w12": np.ascontiguousarray(np.asarray(W12, np.float32)),
            "idx0": br0.idx_all[c],
            "vd0": br0.vd_all[c],
            "idx2": br2.idx_all[c],
            "vd2": br2.vd_all[c],
            "tok": tok,
        })
    return br0, br2, in_maps


def kernel(x_0, x_1, n00_rows, n00_cols, n00_vals, n12_rows, n12_cols, n12_vals,
           W0, W12):
    br0, br2, in_maps = _prepare(x_0, x_1, n00_rows, n00_cols, n00_vals,
                                 n12_rows, n12_cols, n12_vals, W0, W12)
    nc = _build_program(br0, br2)
    res = run_bass_kernel_spmd(nc, in_maps, core_ids=list(range(C)))
    y0 = np.concatenate([res.results[c]["y0"] for c in range(C)], axis=0)
    y1 = np.concatenate([res.results[c]["y1"] for c in range(C)], axis=0)
    y2 = np.concatenate([res.results[c]["y2"] for c in range(C)], axis=0)
    return (y0, y1, y2)


# revision 6
# speedup vs baseline: 1.8760x; 1.8760x over previous
"""Trainium2 Bass kernel for a two-branch sparse message-passing layer (CAN layer).

Computes, given COO edge lists and dense 128x128 weights:
  y0 = relu(spmm(A00, relu(x_0) @ W0))      A00: [100000,100000], 1.6M nnz
  y1 = relu(x_1)                            passthrough [150000,128]
  y2 = relu(spmm(B12, relu(x_1) @ W12))     B12: [50000,150000], 200K nnz

Strategy: shard destination rows across 8 NeuronCores. Each core:
  - gathers source rows for its edges via dma_gather (int16 windowed indices),
  - applies relu on the gathered tiles,
  - aggregates per 128-destination block with one-hot scatter matmuls
    (S[e, d] = (iota_d == dest_local_e) * val_e built on DVE, accumulated
    in fp32 PSUM by the TensorEngine; uses A@(xW) = (A@x)@W associativity so
    the dense weight is applied once per aggregated output block in fp32),
  - applies the dense weight + relu and writes its output shard.

The gather tables and the scatter matmuls run in bf16 (fp32 accumulation);
the x_1 relu passthrough and the dense-weight stage stay fp32.

Host side only shards/organizes the COO lists and concatenates shards.
"""
import math
import numpy as np
import ml_dtypes
from contextlib import ExitStack

import concourse.bass as bass
import concourse.tile as tile
from concourse import bacc, mybir
from concourse.bass_utils import run_bass_kernel_spmd

P = 128
C = 8                       # cores
N0, N1, N2 = 100000, 150000, 50000
R0, R1, R2 = N0 // C, N1 // C, N2 // C      # 12500, 18750, 6250
NB0 = math.ceil(R0 / P)     # 98 dest blocks/core, last has 84 rows
NB2 = math.ceil(R2 / P)     # 49 dest blocks/core, last has 106 rows
SB0 = 4                     # blocks per superblock (gather granularity)
SB2 = 7
W0SZ, NW0 = 25000, 4        # source windows (int16 index limit)
W2SZ, NW2 = 30000, 5
X1CH = 10                   # x1 relu chunks per core
X1W = R1 // X1CH            # 1875 free-dim elements per chunk

USE_BF16 = True             # bf16 gathers + scatter matmuls (fp32 PSUM/dense)

F32 = mybir.dt.float32
BF16 = mybir.dt.bfloat16
I16 = mybir.dt.int16
DT_G = BF16 if USE_BF16 else F32
NP_G = ml_dtypes.bfloat16 if USE_BF16 else np.float32


class _Branch:
    """Host-side slot layout for one SpMM branch (shared across cores)."""

    def __init__(self, rows, cols, vals, R, nb, sbsz, wsz, nw):
        nsb = math.ceil(nb / sbsz)
        core = rows // R
        local = rows - core * R
        b = local // P
        dl = (local % P).astype(np.float32)
        q = cols // wsz
        qcol = (cols % wsz).astype(np.int16)
        sb = b // sbsz
        b_in = b % sbsz

        ncell = nsb * nw * sbsz
        cell = (sb * nw + q) * sbsz + b_in
        key = core * ncell + cell
        cnt = np.bincount(key, minlength=C * ncell).reshape(C, nsb, nw, sbsz)
        g = np.ceil(cnt.max(axis=0) / P).astype(np.int64)   # [nsb, nw, sbsz]

        # group-id offsets within each sb (q-major, then block)
        gflat = g.reshape(nsb, nw * sbsz)
        ggoff = np.zeros((nsb, nw * sbsz), np.int64)
        ggoff[:, 1:] = np.cumsum(gflat, axis=1)[:, :-1]
        ggoff = ggoff.reshape(nsb, nw, sbsz)
        G_sb = gflat.sum(axis=1)                            # groups per sb
        gq = g.sum(axis=2)                                  # [nsb, nw]
        q_goff = ggoff[:, :, 0]                             # first group of (sb,q)

        # flat offsets into the metadata tensors
        ioff = np.zeros((nsb, nw), np.int64)                # int16 elements
        np.cumsum((P * 8 * gq).ravel()[:-1], out=ioff.ravel()[1:])
        TI = int(P * 8 * gq.sum())
        vdoff = np.zeros(nsb, np.int64)
        np.cumsum((P * 2 * G_sb)[:-1], out=vdoff[1:])
        TV = int(P * 2 * G_sb.sum())

        # per-edge slot assignment
        order = np.lexsort((b_in, q, sb, core))
        ks = key[order]
        starts = np.r_[0, np.flatnonzero(np.diff(ks)) + 1]
        seg_len = np.diff(np.r_[starts, len(ks)])
        rank = np.arange(len(ks)) - np.repeat(starts, seg_len)

        core_s, sb_s, q_s, bin_s = core[order], sb[order], q[order], b_in[order]
        p_s = rank % P
        gg_s = ggoff[sb_s, q_s, bin_s] + rank // P
        j = (gg_s - q_goff[sb_s, q_s]) * P + p_s
        gq_s = gq[sb_s, q_s]

        idx_all = np.zeros((C, max(TI, 1)), np.int16)
        vd_all = np.zeros((C, max(TV, 1)), np.float32)
        base_i = ioff[sb_s, q_s]
        colw = j // 16
        r16 = j % 16
        qcol_s = qcol[order]
        for rr in range(8):
            pos = base_i + (16 * rr + r16) * (8 * gq_s) + colw
            idx_all[core_s, pos] = qcol_s
        G_s = G_sb[sb_s]
        posv = vdoff[sb_s] + p_s * (2 * G_s) + gg_s
        vd_all[core_s, posv] = vals[order]
        vd_all[core_s, posv + G_s] = dl[order]

        self.nb, self.sbsz, self.nsb, self.nw, self.wsz = nb, sbsz, nsb, nw, wsz
        self.R = R
        self.g, self.ggoff, self.G_sb, self.gq = g, ggoff, G_sb, gq
        self.ioff, self.vdoff, self.TI, self.TV = ioff, vdoff, TI, TV
        self.idx_all, self.vd_all = idx_all, vd_all

    def sb_blocks(self, s):
        return range(s * self.sbsz, min((s + 1) * self.sbsz, self.nb))


def _emit_branch_sb(nc, ctx, pools, br, s, src_h, idx_h, vd_h, w_t, iota_t, y_h):
    """Emit one superblock of a SpMM branch."""
    xg_pool, meta_pool, s_pool, t_pool, out_pool, ps1, ps2 = pools
    G_sb = int(br.G_sb[s])
    if G_sb == 0:
        return
    vd_t = meta_pool.tile([P, 2 * G_sb], F32, tag=f"vd{br.wsz}")
    nc.sync.dma_start(vd_t[:], bass.AP(vd_h, int(br.vdoff[s]), [[2 * G_sb, P], [1, 2 * G_sb]]))

    xg = xg_pool.tile([P, G_sb * P], DT_G, tag=f"xg{br.wsz}")
    for q in range(br.nw):
        gq = int(br.gq[s, q])
        if gq == 0:
            continue
        nidx = P * gq
        idx_t = meta_pool.tile([P, 8 * gq], I16, tag=f"ix{br.wsz}")
        nc.sync.dma_start(idx_t[:], bass.AP(idx_h, int(br.ioff[s, q]), [[8 * gq, P], [1, 8 * gq]]))
        off = int(br.ggoff[s, q, 0])
        seg = xg[:, off * P:(off + gq) * P]
        nc.gpsimd.dma_gather(
            out_ap=seg.rearrange("p (g e) -> p g e", e=P),
            in_ap=src_h.ap()[q * br.wsz:(q + 1) * br.wsz, :],
            idxs_ap=idx_t[:],
            num_idxs=nidx,
            num_idxs_reg=nidx,
            elem_size=P,
            single_packet=(nidx <= 1024),
        )
        # relu the freshly gathered segment in place
        nc.scalar.activation(seg, seg, mybir.ActivationFunctionType.Relu)

    for b in br.sb_blocks(s):
        b_in = b - s * br.sbsz
        ggs = []
        for q in range(br.nw):
            o = int(br.ggoff[s, q, b_in])
            ggs.extend(range(o, o + int(br.g[s, q, b_in])))
        if not ggs:
            continue
        psum1 = ps1.tile([P, P], F32, space="PSUM", tag="ps1")
        for i, gg in enumerate(ggs):
            s_t = s_pool.tile([P, P], DT_G, tag="s")
            nc.vector.tensor_scalar(
                out=s_t[:], in0=iota_t[:],
                scalar1=vd_t[:, G_sb + gg:G_sb + gg + 1],
                scalar2=vd_t[:, gg:gg + 1],
                op0=mybir.AluOpType.is_equal, op1=mybir.AluOpType.mult,
            )
            nc.tensor.matmul(
                psum1[:], lhsT=xg[:, gg * P:(gg + 1) * P], rhs=s_t[:],
                start=(i == 0), stop=(i == len(ggs) - 1),
            )
        t1 = t_pool.tile([P, P], F32, tag="t1")
        nc.scalar.activation(t1[:], psum1[:], mybir.ActivationFunctionType.Copy)
        psum2 = ps2.tile([P, P], F32, space="PSUM", tag="ps2")
        nc.tensor.matmul(psum2[:], lhsT=t1[:], rhs=w_t[:], start=True, stop=True)
        yt = out_pool.tile([P, P], F32, tag="yt")
        nc.scalar.activation(yt[:], psum2[:], mybir.ActivationFunctionType.Relu)
        r0 = b * P
        nrow = min(br.R, r0 + P) - r0
        nc.sync.dma_start(y_h.ap()[r0:r0 + nrow, :], yt[:nrow, :])


def _build_program(br0, br2, loop_k=None):
    nc = bacc.Bacc("TRN2", target_bir_lowering=False, debug=False, num_devices=C)
    x0_h = nc.dram_tensor("x0", [N0, P], DT_G, kind="ExternalInput")
    x1f_h = nc.dram_tensor("x1f", [N1, P], DT_G, kind="ExternalInput")
    x1s_h = nc.dram_tensor("x1s", [R1, P], F32, kind="ExternalInput")
    w0_h = nc.dram_tensor("w0", [P, P], F32, kind="ExternalInput")
    w12_h = nc.dram_tensor("w12", [P, P], F32, kind="ExternalInput")
    idx0_h = nc.dram_tensor("idx0", [max(br0.TI, 1)], I16, kind="ExternalInput")
    vd0_h = nc.dram_tensor("vd0", [max(br0.TV, 1)], F32, kind="ExternalInput")
    idx2_h = nc.dram_tensor("idx2", [max(br2.TI, 1)], I16, kind="ExternalInput")
    vd2_h = nc.dram_tensor("vd2", [max(br2.TV, 1)], F32, kind="ExternalInput")
    tok_h = nc.dram_tensor("tok", [1, P], F32, kind="ExternalInput")
    y0_h = nc.dram_tensor("y0", [R0, P], F32, kind="ExternalOutput")
    y1_h = nc.dram_tensor("y1", [R1, P], F32, kind="ExternalOutput")
    y2_h = nc.dram_tensor("y2", [R2, P], F32, kind="ExternalOutput")
    tko_h = nc.dram_tensor("tok_out", [1, P], F32, kind="ExternalOutput")

    with tile.TileContext(nc) as tc:
        with ExitStack() as ctx:
            const = ctx.enter_context(tc.tile_pool(name="const", bufs=1))
            xg_pool = ctx.enter_context(tc.tile_pool(name="xg", bufs=2))
            meta_pool = ctx.enter_context(tc.tile_pool(name="meta", bufs=3))
            s_pool = ctx.enter_context(tc.tile_pool(name="s", bufs=4))
            t_pool = ctx.enter_context(tc.tile_pool(name="t1", bufs=3))
            out_pool = ctx.enter_context(tc.tile_pool(name="out", bufs=3))
            x1_pool = ctx.enter_context(tc.tile_pool(name="x1", bufs=3))
            ps1 = ctx.enter_context(tc.tile_pool(name="ps1", bufs=4, space="PSUM"))
            ps2 = ctx.enter_context(tc.tile_pool(name="ps2", bufs=4, space="PSUM"))
            pools = (xg_pool, meta_pool, s_pool, t_pool, out_pool, ps1, ps2)

            # token roundtrip (used by the timing rig to chain executions)
            tok_t = const.tile([1, P], F32)
            nc.sync.dma_start(tok_t[:], tok_h.ap())
            nc.sync.dma_start(tko_h.ap(), tok_t[:])

            iota_t = const.tile([P, P], DT_G)
            nc.gpsimd.iota(iota_t[:], [[1, P]], channel_multiplier=0,
                           allow_small_or_imprecise_dtypes=True)
            w0_t = const.tile([P, P], F32)
            nc.sync.dma_start(w0_t[:], w0_h.ap())
            w12_t = const.tile([P, P], F32)
            nc.sync.dma_start(w12_t[:], w12_h.ap())

            # interleave work units for overlap: b0 superblocks, b2 superblocks,
            # x1 relu chunks
            units = [("b0", s) for s in range(br0.nsb)]
            step0 = max(1, br0.nsb // br2.nsb)
            for i in range(br2.nsb):
                units.insert(min(len(units), (i + 1) * (step0 + 1) - 1), ("b2", i))
            stepx = max(1, len(units) // X1CH)
            for i in range(X1CH):
                units.insert(min(len(units), (i + 1) * (stepx + 1) - 1), ("x1", i))

            def emit_units():
                for kind, s in units:
                    if kind == "b0":
                        _emit_branch_sb(nc, ctx, pools, br0, s, x0_h, idx0_h, vd0_h,
                                        w0_t, iota_t, y0_h)
                    elif kind == "b2":
                        _emit_branch_sb(nc, ctx, pools, br2, s, x1f_h, idx2_h, vd2_h,
                                        w12_t, iota_t, y2_h)
                    else:
                        c = s
                        xt = x1_pool.tile([P, X1W], F32, tag="x1c")
                        src = bass.AP(x1s_h, c * X1W, [[R1, P], [1, X1W]])
                        dst = bass.AP(y1_h, c * X1W, [[R1, P], [1, X1W]])
                        nc.sync.dma_start(xt[:], src)
                        nc.scalar.activation(xt[:], xt[:], mybir.ActivationFunctionType.Relu)
                        nc.sync.dma_start(dst, xt[:])

            if loop_k is None:
                emit_units()
            else:
                with tc.For_i(0, loop_k, 1):
                    emit_units()
    nc.compile()
    return nc


def _prepare(x_0, x_1, n00_rows, n00_cols, n00_vals, n12_rows, n12_cols, n12_vals,
             W0, W12):
    br0 = _Branch(np.asarray(n00_rows), np.asarray(n00_cols),
                  np.asarray(n00_vals, np.float32), R0, NB0, SB0, W0SZ, NW0)
    br2 = _Branch(np.asarray(n12_rows), np.asarray(n12_cols),
                  np.asarray(n12_vals, np.float32), R2, NB2, SB2, W2SZ, NW2)
    x_0g = np.ascontiguousarray(np.asarray(x_0).astype(NP_G))
    x_1 = np.ascontiguousarray(np.asarray(x_1, np.float32))
    x_1g = np.ascontiguousarray(x_1.astype(NP_G))
    tok = np.zeros((1, P), np.float32)
    in_maps = []
    for c in range(C):
        in_maps.append({
            "x0": x_0g,
            "x1f": x_1g,
            "x1s": np.ascontiguousarray(x_1[c * R1:(c + 1) * R1]),
            "w0": np.ascontiguousarray(np.asarray(W0, np.float32)),
            "w12": np.ascontiguousarray(np.asarray(W12, np.float32)),
            "idx0": br0.idx_all[c],
            "vd0": br0.vd_all[c],
            "idx2": br2.idx_all[c],
            "vd2": br2.vd_all[c],
            "tok": tok,
        })
    return br0, br2, in_maps


def kernel(x_0, x_1, n00_rows, n00_cols, n00_vals, n12_rows, n12_cols, n12_vals,
           W0, W12):
    br0, br2, in_maps = _prepare(x_0, x_1, n00_rows, n00_cols, n00_vals,
                                 n12_rows, n12_cols, n12_vals, W0, W12)
    nc = _build_program(br0, br2)
    res = run_bass_kernel_spmd(nc, in_maps, core_ids=list(range(C)))
    y0 = np.concatenate([res.results[c]["y0"] for c in range(C)], axis=0)
    y1 = np.concatenate([res.results[c]["y1"] for c in range(C)], axis=0)
    y2 = np.concatenate([res.results[c]["y2"] for c in range(C)], axis=0)
    return (y0, y1, y2)


# revision 9
# speedup vs baseline: 15.5834x; 8.3068x over previous
"""Trainium2 Bass kernel for a two-branch sparse message-passing layer (CAN layer).

Computes, given COO edge lists and dense 128x128 weights:
  y0 = relu(spmm(A00, relu(x_0) @ W0))      A00: [100000,100000], 1.6M nnz
  y1 = relu(x_1)                            passthrough [150000,128]
  y2 = relu(spmm(B12, relu(x_1) @ W12))     B12: [50000,150000], 200K nnz

Strategy: shard destination rows across 8 NeuronCores. Each core:
  - gathers source rows for its edges via dma_gather (int16 windowed indices),
  - applies relu on the gathered tiles,
  - aggregates per 128-destination block with one-hot scatter matmuls
    (S[e, d] = (iota_d == dest_local_e) * val_e built on DVE, accumulated
    in fp32 PSUM by the TensorEngine; uses A@(xW) = (A@x)@W associativity so
    the dense weight is applied once per aggregated output block in fp32),
  - applies the dense weight + relu and writes its output shard.

The gather tables and the scatter matmuls run in bf16 (fp32 accumulation);
the x_1 relu passthrough and the dense-weight stage stay fp32.

Host side only shards/organizes the COO lists and concatenates shards.
"""
import math
import numpy as np
import ml_dtypes
from contextlib import ExitStack

import concourse.bass as bass
import concourse.tile as tile
from concourse import bacc, mybir
from concourse.bass_utils import run_bass_kernel_spmd

P = 128
C = 8                       # cores
N0, N1, N2 = 100000, 150000, 50000
R0, R1, R2 = N0 // C, N1 // C, N2 // C      # 12500, 18750, 6250
NB0 = math.ceil(R0 / P)     # 98 dest blocks/core, last has 84 rows
NB2 = math.ceil(R2 / P)     # 49 dest blocks/core, last has 106 rows
SB0 = 4                     # blocks per superblock (gather granularity)
SB2 = 7
W0SZ, NW0 = 25000, 4        # source windows (int16 index limit)
W2SZ, NW2 = 30000, 5
X1CH = 10                   # x1 relu chunks per core
X1W = R1 // X1CH            # 1875 free-dim elements per chunk

USE_BF16 = True             # bf16 gathers + scatter matmuls (fp32 PSUM/dense)

import os
SKIP_GATHER = bool(int(os.environ.get("K_SKIP_GATHER", "0")))
SKIP_COMPUTE = bool(int(os.environ.get("K_SKIP_COMPUTE", "0")))
SKIP_X1 = bool(int(os.environ.get("K_SKIP_X1", "0")))

F32 = mybir.dt.float32
BF16 = mybir.dt.bfloat16
I16 = mybir.dt.int16
DT_G = BF16 if USE_BF16 else F32
NP_G = ml_dtypes.bfloat16 if USE_BF16 else np.float32


class _Branch:
    """Host-side slot layout for one SpMM branch (shared across cores)."""

    def __init__(self, rows, cols, vals, R, nb, sbsz, wsz, nw):
        nsb = math.ceil(nb / sbsz)
        core = rows // R
        local = rows - core * R
        b = local // P
        dl = (local % P).astype(np.float32)
        q = cols // wsz
        qcol = (cols % wsz).astype(np.int16)
        sb = b // sbsz
        b_in = b % sbsz

        ncell = nsb * nw * sbsz
        cell = (sb * nw + q) * sbsz + b_in
        key = core * ncell + cell
        cnt = np.bincount(key, minlength=C * ncell).reshape(C, nsb, nw, sbsz)
        g = np.ceil(cnt.max(axis=0) / P).astype(np.int64)   # [nsb, nw, sbsz]

        # group-id offsets within each sb (q-major, then block)
        gflat = g.reshape(nsb, nw * sbsz)
        ggoff = np.zeros((nsb, nw * sbsz), np.int64)
        ggoff[:, 1:] = np.cumsum(gflat, axis=1)[:, :-1]
        ggoff = ggoff.reshape(nsb, nw, sbsz)
        G_sb = gflat.sum(axis=1)                            # groups per sb
        gq = g.sum(axis=2)                                  # [nsb, nw]
        q_goff = ggoff[:, :, 0]                             # first group of (sb,q)

        # flat offsets into the metadata tensors
        ioff = np.zeros((nsb, nw), np.int64)                # int16 elements
        np.cumsum((P * 8 * gq).ravel()[:-1], out=ioff.ravel()[1:])
        TI = int(P * 8 * gq.sum())
        vdoff = np.zeros(nsb, np.int64)
        np.cumsum((P * 2 * G_sb)[:-1], out=vdoff[1:])
        TV = int(P * 2 * G_sb.sum())

        # per-edge slot assignment
        order = np.lexsort((b_in, q, sb, core))
        ks = key[order]
        starts = np.r_[0, np.flatnonzero(np.diff(ks)) + 1]
        seg_len = np.diff(np.r_[starts, len(ks)])
        rank = np.arange(len(ks)) - np.repeat(starts, seg_len)

        core_s, sb_s, q_s, bin_s = core[order], sb[order], q[order], b_in[order]
        p_s = rank % P
        gg_s = ggoff[sb_s, q_s, bin_s] + rank // P
        j = (gg_s - q_goff[sb_s, q_s]) * P + p_s
        gq_s = gq[sb_s, q_s]

        idx_all = np.zeros((C, max(TI, 1)), np.int16)
        vd_all = np.zeros((C, max(TV, 1)), np.float32)
        base_i = ioff[sb_s, q_s]
        colw = j // 16
        r16 = j % 16
        qcol_s = qcol[order]
        for rr in range(8):
            pos = base_i + (16 * rr + r16) * (8 * gq_s) + colw
            idx_all[core_s, pos] = qcol_s
        G_s = G_sb[sb_s]
        posv = vdoff[sb_s] + p_s * (2 * G_s) + gg_s
        vd_all[core_s, posv] = vals[order]
        vd_all[core_s, posv + G_s] = dl[order]

        self.nb, self.sbsz, self.nsb, self.nw, self.wsz = nb, sbsz, nsb, nw, wsz
        self.R = R
        self.g, self.ggoff, self.G_sb, self.gq = g, ggoff, G_sb, gq
        self.ioff, self.vdoff, self.TI, self.TV = ioff, vdoff, TI, TV
        self.idx_all, self.vd_all = idx_all, vd_all

    def sb_blocks(self, s):
        return range(s * self.sbsz, min((s + 1) * self.sbsz, self.nb))


def _emit_branch_sb(nc, ctx, pools, br, s, src_h, idx_h, vd_h, w_t, iota_t, y_h):
    """Emit one superblock of a SpMM branch."""
    xg_pool, meta_pool, s_pool, t_pool, out_pool, ps1, ps2 = pools
    G_sb = int(br.G_sb[s])
    if G_sb == 0:
        return
    vd_t = meta_pool.tile([P, 2 * G_sb], F32, tag=f"vd{br.wsz}")
    nc.sync.dma_start(vd_t[:], bass.AP(vd_h, int(br.vdoff[s]), [[2 * G_sb, P], [1, 2 * G_sb]]))

    xg = xg_pool.tile([P, G_sb * P], DT_G, tag=f"xg{br.wsz}")
    for q in range(br.nw):
        gq = int(br.gq[s, q])
        if gq == 0:
            continue
        nidx = P * gq
        idx_t = meta_pool.tile([P, 8 * gq], I16, tag=f"ix{br.wsz}")
        nc.sync.dma_start(idx_t[:], bass.AP(idx_h, int(br.ioff[s, q]), [[8 * gq, P], [1, 8 * gq]]))
        off = int(br.ggoff[s, q, 0])
        seg = xg[:, off * P:(off + gq) * P]
        if not SKIP_GATHER:
            nc.gpsimd.dma_gather(
                out_ap=seg.rearrange("p (g e) -> p g e", e=P),
                in_ap=src_h.ap()[q * br.wsz:(q + 1) * br.wsz, :],
                idxs_ap=idx_t[:],
                num_idxs=nidx,
                num_idxs_reg=nidx,
                elem_size=P,
                single_packet=(nidx <= 1024),
            )
        # relu the freshly gathered segment in place
        nc.scalar.activation(seg, seg, mybir.ActivationFunctionType.Relu)

    if SKIP_COMPUTE:
        return
    for b in br.sb_blocks(s):
        b_in = b - s * br.sbsz
        ggs = []
        for q in range(br.nw):
            o = int(br.ggoff[s, q, b_in])
            ggs.extend(range(o, o + int(br.g[s, q, b_in])))
        if not ggs:
            continue
        psum1 = ps1.tile([P, P], F32, space="PSUM", tag="ps1")
        for i, gg in enumerate(ggs):
            s_t = s_pool.tile([P, P], DT_G, tag="s")
            nc.vector.tensor_scalar(
                out=s_t[:], in0=iota_t[:],
                scalar1=vd_t[:, G_sb + gg:G_sb + gg + 1],
                scalar2=vd_t[:, gg:gg + 1],
                op0=mybir.AluOpType.is_equal, op1=mybir.AluOpType.mult,
            )
            nc.tensor.matmul(
                psum1[:], lhsT=xg[:, gg * P:(gg + 1) * P], rhs=s_t[:],
                start=(i == 0), stop=(i == len(ggs) - 1),
            )
        t1 = t_pool.tile([P, P], F32, tag="t1")
        nc.scalar.activation(t1[:], psum1[:], mybir.ActivationFunctionType.Copy)
        psum2 = ps2.tile([P, P], F32, space="PSUM", tag="ps2")
        nc.tensor.matmul(psum2[:], lhsT=t1[:], rhs=w_t[:], start=True, stop=True)
        yt = out_pool.tile([P, P], F32, tag="yt")
        nc.scalar.activation(yt[:], psum2[:], mybir.ActivationFunctionType.Relu)
        r0 = b * P
        nrow = min(br.R, r0 + P) - r0
        nc.sync.dma_start(y_h.ap()[r0:r0 + nrow, :], yt[:nrow, :])


def _build_program(br0, br2, loop_k=None):
    nc = bacc.Bacc("TRN2", target_bir_lowering=False, debug=False, num_devices=C)
    x0_h = nc.dram_tensor("x0", [N0, P], DT_G, kind="ExternalInput")
    x1f_h = nc.dram_tensor("x1f", [N1, P], DT_G, kind="ExternalInput")
    x1s_h = nc.dram_tensor("x1s", [R1, P], F32, kind="ExternalInput")
    w0_h = nc.dram_tensor("w0", [P, P], F32, kind="ExternalInput")
    w12_h = nc.dram_tensor("w12", [P, P], F32, kind="ExternalInput")
    idx0_h = nc.dram_tensor("idx0", [max(br0.TI, 1)], I16, kind="ExternalInput")
    vd0_h = nc.dram_tensor("vd0", [max(br0.TV, 1)], F32, kind="ExternalInput")
    idx2_h = nc.dram_tensor("idx2", [max(br2.TI, 1)], I16, kind="ExternalInput")
    vd2_h = nc.dram_tensor("vd2", [max(br2.TV, 1)], F32, kind="ExternalInput")
    tok_h = nc.dram_tensor("tok", [1, P], F32, kind="ExternalInput")
    y0_h = nc.dram_tensor("y0", [R0, P], F32, kind="ExternalOutput")
    y1_h = nc.dram_tensor("y1", [R1, P], F32, kind="ExternalOutput")
    y2_h = nc.dram_tensor("y2", [R2, P], F32, kind="ExternalOutput")
    tko_h = nc.dram_tensor("tok_out", [1, P], F32, kind="ExternalOutput")

    with tile.TileContext(nc) as tc:
        with ExitStack() as ctx:
            const = ctx.enter_context(tc.tile_pool(name="const", bufs=1))
            xg_pool = ctx.enter_context(tc.tile_pool(name="xg", bufs=2))
            meta_pool = ctx.enter_context(tc.tile_pool(name="meta", bufs=3))
            s_pool = ctx.enter_context(tc.tile_pool(name="s", bufs=4))
            t_pool = ctx.enter_context(tc.tile_pool(name="t1", bufs=3))
            out_pool = ctx.enter_context(tc.tile_pool(name="out", bufs=3))
            x1_pool = ctx.enter_context(tc.tile_pool(name="x1", bufs=3))
            ps1 = ctx.enter_context(tc.tile_pool(name="ps1", bufs=4, space="PSUM"))
            ps2 = ctx.enter_context(tc.tile_pool(name="ps2", bufs=4, space="PSUM"))
            pools = (xg_pool, meta_pool, s_pool, t_pool, out_pool, ps1, ps2)

            # token roundtrip (used by the timing rig to chain executions)
            tok_t = const.tile([1, P], F32)
            nc.sync.dma_start(tok_t[:], tok_h.ap())
            nc.sync.dma_start(tko_h.ap(), tok_t[:])

            iota_t = const.tile([P, P], DT_G)
            nc.gpsimd.iota(iota_t[:], [[1, P]], channel_multiplier=0,
                           allow_small_or_imprecise_dtypes=True)
            w0_t = const.tile([P, P], F32)
            nc.sync.dma_start(w0_t[:], w0_h.ap())
            w12_t = const.tile([P, P], F32)
            nc.sync.dma_start(w12_t[:], w12_h.ap())

            # interleave work units for overlap: b0 superblocks, b2 superblocks,
            # x1 relu chunks
            units = [("b0", s) for s in range(br0.nsb)]
            step0 = max(1, br0.nsb // br2.nsb)
            for i in range(br2.nsb):
                units.insert(min(len(units), (i + 1) * (step0 + 1) - 1), ("b2", i))
            stepx = max(1, len(units) // X1CH)
            for i in range(X1CH):
                units.insert(min(len(units), (i + 1) * (stepx + 1) - 1), ("x1", i))

            def emit_units():
                for kind, s in units:
                    if kind == "b0":
                        _emit_branch_sb(nc, ctx, pools, br0, s, x0_h, idx0_h, vd0_h,
                                        w0_t, iota_t, y0_h)
                    elif kind == "b2":
                        _emit_branch_sb(nc, ctx, pools, br2, s, x1f_h, idx2_h, vd2_h,
                                        w12_t, iota_t, y2_h)
                    elif not SKIP_X1:
                        c = s
                        xt = x1_pool.tile([P, X1W], F32, tag="x1c")
                        src = bass.AP(x1s_h, c * X1W, [[R1, P], [1, X1W]])
                        dst = bass.AP(y1_h, c * X1W, [[R1, P], [1, X1W]])
                        nc.sync.dma_start(xt[:], src)
                        nc.scalar.activation(xt[:], xt[:], mybir.ActivationFunctionType.Relu)
                        nc.sync.dma_start(dst, xt[:])

            if loop_k is None:
                emit_units()
            else:
                with tc.For_i(0, loop_k, 1):
                    emit_units()
    nc.compile()
    return nc


def _prepare(x_0, x_1, n00_rows, n00_cols, n00_vals, n12_rows, n12_cols, n12_vals,
             W0, W12):
    br0 = _Branch(np.asarray(n00_rows), np.asarray(n00_cols),
                  np.asarray(n00_vals, np.float32), R0, NB0, SB0, W0SZ, NW0)
    br2 = _Branch(np.asarray(n12_rows), np.asarray(n12_cols),
                  np.asarray(n12_vals, np.float32), R2, NB2, SB2, W2SZ, NW2)
    x_0g = np.ascontiguousarray(np.asarray(x_0).astype(NP_G))
    x_1 = np.ascontiguousarray(np.asarray(x_1, np.float32))
    x_1g = np.ascontiguousarray(x_1.astype(NP_G))
    tok = np.zeros((1, P), np.float32)
    in_maps = []
    for c in range(C):
        in_maps.append({
            "x0": x_0g,
            "x1f": x_1g,
            "x1s": np.ascontiguousarray(x_1[c * R1:(c + 1) * R1]),
            "w0": np.ascontiguousarray(np.asarray(W0, np.float32)),
            "w12": np.ascontiguousarray(np.asarray(W12, np.float32)),
            "idx0": br0.idx_all[c],
            "vd0": br0.vd_all[c],
            "idx2": br2.idx_all[c],
            "vd2": br2.vd_all[c],
            "tok": tok,
        })
    return br0, br2, in_maps


def kernel(x_0, x_1, n00_rows, n00_cols, n00_vals, n12_rows, n12_cols, n12_vals,
           W0, W12):
    br0, br2, in_maps = _prepare(x_0, x_1, n00_rows, n00_cols, n00_vals,
                                 n12_rows, n12_cols, n12_vals, W0, W12)
    nc = _build_program(br0, br2)
    res = run_bass_kernel_spmd(nc, in_maps, core_ids=list(range(C)))
    y0 = np.concatenate([res.results[c]["y0"] for c in range(C)], axis=0)
    y1 = np.concatenate([res.results[c]["y1"] for c in range(C)], axis=0)
    y2 = np.concatenate([res.results[c]["y2"] for c in range(C)], axis=0)
    return (y0, y1, y2)
